# revision 74
# baseline (speedup 1.0000x reference)
"""GAT (graph attention) Trainium2 kernel.

Full-input contract: kernel(**inputs) takes the unsharded tensors
  x   (8, 1024, 512) f32
  adj (8, 1024, 1024) i32
  W   (8, 256, 512) f32
  a1  (8, 256) f32
  a2  (8, 256) f32
and returns out (8, 1024, 256) f32.

Sharding: data-parallel over batch B=8 across the 8 NeuronCores; each core
computes all heads for one batch element. No collectives needed.

Attention restructuring (v2, exact math): with v = f1_i + f2_j,
  exp(lrelu(v)) = max(e^v, e^{0.2v}) = e^{0.2 f1_i} * [ e^{0.2 f2_j} *
                  max(e^{0.8 (f1_i + f2_j)}, 1) ]
and the e^{0.2 f1_i} row factor cancels in the softmax normalization, so
the unnormalized attention only needs
  t_ij = adj_ij * b2_j * max(c_i * d_j, 1)
with c = exp(0.8 f1) (bf16 row, partition-broadcast), d = exp(0.8 f2) and
b2 = exp(0.2 f2) per-partition scalars. Per (head, node-tile) this is ONE
DVE tensor_scalar (4x mode: u = max(cb*d, 1)) plus ONE adjacency-mask
multiply; b2 is folded into the stage-B PSUM evacuation (tensor_scalar
mult instead of plain copy) and into the denominator ones-column.

Structure (pipelined): DMA order a12 -> x -> W -> adj. W is processed in
head pairs as it arrives (w12, WT transposes, stage-B passes 0-1 with
plain-copy evacs — no f12 dependency); passes 2-3 (heads 4-7, with the b2
fold in the evac) are spread one group per epilogue slot across the early
heads. Head 0's attention is column-paced: its mask-muls and epilogues
start per 128-column block as each adj row-tile is cast + transposed, so
the stream begins during the adj DMA tail. Heads 0-3 apply b2 via
per-head stream tensor_scalars instead of the evac fold. All activations
are pinned to the natural_log_exp_and_others hardware table set (one
LoadActFuncSet total, no exp<->ln thrash at the log-softmax finale).

  h_h   = x @ W_h^T                  (bf16 PE matmul, fp32 accum)
  f1/f2 = x @ (W_h^T a)              (bf16 inputs, fp32 accum)
  o = att @ [b2*h | b2]              (PE; b2-scaled ones col = denom)
  elu(o/den) + 1 = min(exp(o/den),1) + max(o/den,0): exp on ACT, relu on
      ACT/DVE, min-clamp on Pool; the head-sum accumulates on PE via
      identity matmuls into PSUM (one accumulation chain per bank, so
      epilogue tiles are paired to full bank width)
  out = log_softmax(sum_h ...)       finale in three stages (its 0-3 at
      it=3, 4-5 at it=5, 6-7 at the end) with separate ds/lnd tiles per
      stage and per-stage output DMAs on the ACT-issued HWDGE queue
"""
import sys

sys.path.insert(0, "/opt/trn_rl_repo")

from contextlib import ExitStack

import numpy as np

import concourse.bacc as bacc
import concourse.bass as bass
import concourse.mybir as mybir
import concourse.tile as tile
from concourse import masks
from concourse._compat import with_exitstack

# Pin every activation to the one hardware table set that holds all four
# functions we use (exp, relu, copy, ln): empty the other sets so the
# table-load pass must choose it, giving ONE LoadActFuncSet for the whole
# program instead of thrashing between the exp and ln sets around the
# log-softmax finale. Set indices are preserved, so the emitted
# act_func_set_id still matches act_info.json.
_PINNED_ACT_SET = "natural_log_exp_and_others"


def _pin_activation_tables():
    import concourse.hw_specs as hw_specs
    if getattr(bacc, "_gat_act_pin", None):
        return
    orig = bacc.get_activation_tables

    def pinned(arch):
        tabs = orig(arch)
        if _PINNED_ACT_SET not in tabs:
            return tabs
        return {k: (v if k == _PINNED_ACT_SET else set()) for k, v in tabs.items()}

    bacc.get_activation_tables = pinned
    hw_specs.get_activation_tables = pinned
    bacc._gat_act_pin = True

F32 = mybir.dt.float32
BF16 = mybir.dt.bfloat16
I32 = mybir.dt.int32
AF = mybir.ActivationFunctionType
ALU = mybir.AluOpType

N, F_IN, F_OUT, H, B = 1024, 512, 256, 8, 8
P = 128
NT = N // P        # 8 node tiles
FT = F_IN // P     # 4 f_in tiles
OT = F_OUT // P    # 2 f_out tiles
HB = F_OUT + 2     # per-head block in hb_all: 256 values + denom col + pad


@with_exitstack
def gat_kernel(ctx: ExitStack, tc, out_d, x_d, adj_d, W_d, a1_d, a2_d,
               variant=()):
    nc = tc.nc
    cfg = {k.split("=")[0]: int(k.split("=")[1]) for k in variant if "=" in k}
    att_pool = cfg.get("attpool", 0)   # per head: trailing att muls on Pool
    rt_dve = cfg.get("rtdve", 1)       # trailing heads whose rt runs on DVE
    rt_pool = cfg.get("rtpool", 0)     # heads >= H - rtpool: rt on Pool
    adj_act = cfg.get("adjact", 1)     # half the adjT evacs on ACT

    const = ctx.enter_context(tc.tile_pool(name="const", bufs=1))
    ident = const.tile([P, P], F32, name="ident", tag="ident")
    masks.make_identity(nc, ident[:])
    ident_bf = const.tile([P, P], BF16, name="ident_bf", tag="ident_bf")
    masks.make_identity(nc, ident_bf[:])

    persist = ctx.enter_context(tc.tile_pool(name="persist", bufs=1))
    xT_bf = [persist.tile([P, N], BF16, name=f"xTbf{fc}", tag=f"xTbf{fc}") for fc in range(FT)]
    WT_bf = [persist.tile([P, H * F_OUT], BF16, name=f"WTbf{fc}", tag=f"WTbf{fc}") for fc in range(FT)]
    hb_all = [persist.tile([P, H * HB], BF16, name=f"hball{nt}", tag=f"hball{nt}") for nt in range(NT)]
    adjT = persist.tile([P, NT * N], BF16, name="adjT", tag="adjT")
    adjTv = adjT[:].rearrange("p (jt i) -> p jt i", jt=NT)
    f12 = [persist.tile([P, 16], F32, name=f"f12_{nt}", tag=f"f12_{nt}") for nt in range(NT)]
    # db[nt]: cols 0:8 = d = exp(0.8 f2) per head, cols 8:16 = b2 = exp(0.2 f2)
    db = [persist.tile([P, 16], F32, name=f"db_{nt}", tag=f"db_{nt}") for nt in range(NT)]
    e1flat = persist.tile([1, H * N], BF16, name="e1flat", tag="e1flat")

    # pools that outlive stage A must be created before it (LIFO closing)
    pj = ctx.enter_context(tc.tile_pool(name="adjload", bufs=cfg.get("pjb", 5)))
    pjb = ctx.enter_context(tc.tile_pool(name="adjcast", bufs=8))

    # ps_h (frontend stage-B PSUM) lives in its own stack closed right after
    # stage A so the stream region gets its banks back
    psh_stack = ctx.enter_context(ExitStack())
    ps_h = psh_stack.enter_context(
        tc.tile_pool(name="psH", bufs=cfg.get("psh", 2), space="PSUM"))

    sa = ctx.enter_context(ExitStack())  # stage-A transients, closed manually
    pa = sa.enter_context(tc.tile_pool(name="stageA", bufs=8))
    pa2 = sa.enter_context(tc.tile_pool(name="stageA2", bufs=16))
    xtf_pool = sa.enter_context(tc.tile_pool(name="xtf", bufs=1))
    ps_a = sa.enter_context(tc.tile_pool(name="psA", bufs=2, space="PSUM"))
    ps_ft = sa.enter_context(tc.tile_pool(name="psFt", bufs=1, space="PSUM"))
    ps_aw = sa.enter_context(tc.tile_pool(name="psAw", bufs=1, space="PSUM"))
    ps_af = sa.enter_context(tc.tile_pool(name="psAf", bufs=2, space="PSUM"))

    w12_sb = xtf_pool.tile([P, 64], F32, name="w12", tag="w12")
    a12_sb = xtf_pool.tile([16, F_OUT], F32, name="a12", tag="a12")

    # ---- DMA queue order: a1/a2, x, W, adj ----
    nc.sync.dma_start(a12_sb[0:8, :], a1_d[:, :])
    nc.sync.dma_start(a12_sb[8:16, :], a2_d[:, :])

    xnats = []
    for nt in range(NT):
        xnat = pa.tile([P, F_IN], F32, name="xnat", tag="xnat")
        nc.sync.dma_start(xnat[:], x_d[nt * P:(nt + 1) * P, :])
        xnats.append(xnat)

    a12T = xtf_pool.tile([P, 32], F32, name="a12T", tag="a12T")
    for ot in range(OT):
        pt = ps_af.tile([P, 16], F32, name="psA_f", tag="psA_f")
        nc.tensor.matmul(pt[:], a12_sb[:, ot * P:(ot + 1) * P],
                         ident[0:16, 0:16], is_transpose=True)
        nc.vector.tensor_copy(a12T[:, ot * 16:(ot + 1) * 16], pt[:])
    a12Tv = a12T[:].rearrange("p (t c h) -> p t c h", t=2, c=2)

    # x transposes (PE, f32) with a single DVE cast-evac per group
    for ntq in range(0, NT, 4):
        for fc in range(FT):
            pt = ps_a.tile([P, 4 * P], F32, name="psA", tag="psA")
            for d in range(4):
                nc.tensor.matmul(pt[:, d * P:(d + 1) * P],
                                 xnats[ntq + d][:, fc * P:(fc + 1) * P],
                                 ident[:], is_transpose=True)
            nc.vector.tensor_copy(xT_bf[fc][:, ntq * P:(ntq + 4) * P], pt[:])

    def stage_b_group(hp, nt, pool=None, fold=False, fold_pool=False):
        """x @ W for head pair hp, node tile nt.

        fold=True (stream passes, heads 4-7): evac applies the b2 scale per
        head (two tensor_scalars split across DVE/ACT). fold=False (frontend
        passes, heads 0-3): plain pair-wide copy — b2 is applied later by
        per-head stream ops, so these evacs have no f12 dependency and can
        run as soon as W arrives.
        """
        hps = (pool or ps_h).tile([P, 2 * F_OUT], F32, name="hpsum", tag="hpsum")
        for fc in range(FT):
            nc.tensor.matmul(hps[:], xT_bf[fc][:, nt * P:(nt + 1) * P],
                             WT_bf[fc][:, hp * 2 * F_OUT:(hp + 1) * 2 * F_OUT],
                             start=(fc == 0), stop=(fc == FT - 1))
        hv = hb_all[nt][:].rearrange("p (h c) -> p h c", h=H)
        src = hps[:].rearrange("p (d c) -> p d c", d=2)
        if not fold:
            # frontend evacs all on DVE: keeps ACT free for the WT evacs
            # and the e1r chain that gates cb0 / head 0
            nc.vector.tensor_copy(hv[:, 2 * hp:2 * hp + 2, 0:F_OUT], src)
            return
        for dh in range(2):
            h = 2 * hp + dh
            if fold_pool:
                nc.gpsimd.tensor_scalar(hv[:, h, 0:F_OUT], src[:, dh],
                                        db[nt][:, 8 + h:9 + h], None,
                                        op0=ALU.mult)
            elif (dh + nt) % 2 == 0:
                nc.vector.tensor_scalar(hv[:, h, 0:F_OUT], src[:, dh],
                                        db[nt][:, 8 + h:9 + h], None,
                                        op0=ALU.mult)
            else:
                nc.scalar.activation(hv[:, h, 0:F_OUT], src[:, dh],
                                     AF.Copy, scale=db[nt][:, 8 + h:9 + h])

    # softmax denominator columns for heads 0-3: plain ones (the per-head
    # stream tensor_scalar turns them into b2); no dependencies, emit first
    for nt in range(NT):
        hv = hb_all[nt][:].rearrange("p (h c) -> p h c", h=H)
        nc.vector.memset(hv[:, 0:4, F_OUT:F_OUT + 1], 1.0)

    # W pair blocks: loads, w12 = W^T @ [a1 a2], WT transposes, and the
    # frontend stage-B passes (pairs 0-1), all paced by W arrival
    w12v = w12_sb[:].rearrange("p (fc c h) -> p fc c h", fc=FT, c=2)
    wnat_all = {}
    for hp in range(4):
        for dh in range(2):
            h = 2 * hp + dh
            wp = ps_aw.tile([P, 8], F32, name="psA_w", tag="psA_w")
            wnats = []
            for ot in range(OT):
                wnat = pa2.tile([P, F_IN], F32, name="wnat", tag="wnat")
                wnats.append(wnat)
                nc.sync.dma_start(wnat[:], W_d[h, ot * P:(ot + 1) * P, :])
            wnat_all[h] = wnats
            for fc in range(FT):
                for ot in range(OT):
                    nc.tensor.matmul(
                        wp[:, fc * 2:(fc + 1) * 2],
                        wnats[ot][:, fc * P:(fc + 1) * P],
                        a12Tv[:, ot, :, h],
                        start=(ot == 0), stop=(ot == OT - 1))
            nc.vector.tensor_copy(w12v[:, :, :, h], wp[:].rearrange("p (fc c) -> p fc c", fc=FT))
        # WT transposes for this pair (PE), ACT evac
        for fc in range(FT):
            pt = ps_a.tile([P, 4 * P], F32, name="psA", tag="psA")
            for dh in range(2):
                for ot in range(OT):
                    nc.tensor.matmul(
                        pt[:, (dh * 2 + ot) * P:(dh * 2 + ot + 1) * P],
                        wnat_all[2 * hp + dh][ot][:, fc * P:(fc + 1) * P],
                        ident[:], is_transpose=True)
            nc.scalar.copy(
                WT_bf[fc][:, 2 * hp * F_OUT:(2 * hp + 2) * F_OUT], pt[:])
        if hp == 0:
            for nt in range(NT):
                stage_b_group(hp, nt)

    w12bf = xtf_pool.tile([P, 64], BF16, name="w12bf", tag="w12bf")
    nc.vector.tensor_copy(w12bf[:], w12_sb[:])
    w12bv = w12bf[:].rearrange("p (fc c h) -> p fc c h", fc=FT, c=2)

    # adj loads (DMA queue ordered after x and W); casts emitted on Pool
    adj_i32 = []
    for it in range(NT):
        ai = pj.tile([P, N], I32, name="adji", tag="adji")
        nc.sync.dma_start(ai[:], adj_d[it * P:(it + 1) * P, :])
        adj_i32.append(ai)

    # f1/f2 = x @ w12 (bf16 inputs, f32 accum); exp(0.8 f1) rows -> e1flat.
    # Two passes so the ft transposes don't head-of-line block PE on each
    # nt's DVE/ACT chain.
    for nt in range(NT):
        fp = ps_af.tile([P, 16], F32, name="psA_f", tag="psA_f")
        for fc in range(FT):
            nc.tensor.matmul(fp[:], xT_bf[fc][:, nt * P:(nt + 1) * P],
                             w12bv[:, fc], start=(fc == 0), stop=(fc == FT - 1))
        nc.vector.tensor_copy(f12[nt][:], fp[:])
        # d = exp(0.8 f2), b2 = exp(0.2 f2) per-partition scalars (ACT, tiny)
        nc.scalar.activation(db[nt][:, 0:8], f12[nt][:, 8:16], AF.Exp, scale=0.8)
        nc.scalar.activation(db[nt][:, 8:16], f12[nt][:, 8:16], AF.Exp, scale=0.2)
        # denominator columns of heads 4-7 = b2 (their evacs fold b2)
        hv = hb_all[nt][:].rearrange("p (h c) -> p h c", h=H)
        nc.vector.tensor_copy(
            hv[:, 4:8, F_OUT:F_OUT + 1],
            db[nt][:, 12:16].rearrange("p (h c) -> p h c", c=1))
    # f1 transposes + exp, batched 4 node tiles per PSUM bank so the
    # latency chain to e1flat is two rounds, not eight; stage-B pass 1
    # groups fill the PE gaps between rounds
    for q in range(2):
        ft = ps_ft.tile([16, 4 * P], F32, name="psA_ft", tag="psA_ft")
        for d in range(4):
            nc.tensor.matmul(ft[:, d * P:(d + 1) * P], f12[4 * q + d][:],
                             ident[:], is_transpose=True)
        e1r = pa.tile([8, 4 * P], BF16, name="e1r", tag="e1r")
        nc.scalar.activation(e1r[:], ft[0:8, :], AF.Exp, scale=0.8)
        # gather via the ACT-issued HWDGE queue: keeps the SP queue free
        # for the bulk input stream
        nc.scalar.dma_start(
            e1flat[0:1, :].rearrange("a (h n) -> a h n", h=H)
            [:, :, 4 * q * P:4 * (q + 1) * P],
            e1r[:])
        for nt in range(4 * q, 4 * q + 4):
            stage_b_group(1, nt)

    # close stage-A transients, then the frontend stage-B PSUM
    sa.close()
    psh_stack.close()

    # Pool queue: first two adj casts (unblocks the first adjT transposes),
    # then the first head broadcast, then the rest interleaved.
    cbp = ctx.enter_context(tc.tile_pool(name="cbp", bufs=2))
    cb_pre = {}
    adj_bf = []

    def emit_adjb(it):
        ab = pjb.tile([P, N], BF16, name="adjb", tag="adjb")
        nc.gpsimd.tensor_scalar(ab[:], adj_i32[it][:], 0, None, op0=ALU.add)
        adj_bf.append(ab)

    def emit_cb(h):
        cb = cbp.tile([P, N], BF16, name="cb", tag="cb")
        nc.gpsimd.partition_broadcast(cb[:], e1flat[0:1, h * N:(h + 1) * N])
        cb_pre[h] = cb

    # Pool order: four casts (DMA-paced; they gate the adjT transposes),
    # cb0 (gates head 0's u ops; e1flat lands ~when cast 3 finishes),
    # remaining casts, cb1
    for it in range(0, 4):
        emit_adjb(it)
    emit_cb(0)
    for it in range(4, NT):
        emit_adjb(it)
    emit_cb(1)

    # ---------------- stage-C pools ----------------
    # bank budget: ps_t 1 + ps_h2 1 + ps_o 2 + ps_s 4 = 8
    ps_t = ctx.enter_context(tc.tile_pool(name="psT", bufs=1, space="PSUM"))
    ps_h2 = ctx.enter_context(tc.tile_pool(name="psH2", bufs=1, space="PSUM"))
    ps_o = ctx.enter_context(tc.tile_pool(name="psO", bufs=cfg.get("pso", 2), space="PSUM"))
    ps_s = ctx.enter_context(tc.tile_pool(name="psS", bufs=1, space="PSUM"))
    tmp_p = ctx.enter_context(tc.tile_pool(name="tmp", bufs=cfg.get("tmpb", 11)))
    att_p = ctx.enter_context(tc.tile_pool(name="attp", bufs=cfg.get("attb", 13)))
    hbp = ctx.enter_context(tc.tile_pool(name="hbp", bufs=cfg.get("hbb", 16)))
    ep = ctx.enter_context(tc.tile_pool(name="epilogue", bufs=cfg.get("epb", 6)))
    dp = ctx.enter_context(tc.tile_pool(name="lsm", bufs=2))

    def adjt_pass(its):
        for it in its:
            for jq in range(0, NT, 4):
                pt = ps_t.tile([P, 4 * P], BF16, name="psT", tag="psT")
                for d in range(4):
                    nc.tensor.matmul(pt[:, d * P:(d + 1) * P],
                                     adj_bf[it][:, (jq + d) * P:(jq + d + 1) * P],
                                     ident_bf[:], is_transpose=True)
                dst = adjTv[:, jq:jq + 4, it * P:(it + 1) * P]
                src = pt[:].rearrange("p (d c) -> p d c", d=4)
                if (it + jq) % 2 == 0 and adj_act:
                    nc.scalar.copy(dst, src)
                else:
                    nc.vector.tensor_copy(dst, src)

    def head_prep(h):
        """cb broadcast, per-head b2 application (heads 0-3), u rows."""
        if h in cb_pre:
            cb = cb_pre.pop(h)
        else:
            cb = cbp.tile([P, N], BF16, name="cb", tag="cb")
            nc.gpsimd.partition_broadcast(cb[:], e1flat[0:1, h * N:(h + 1) * N])
        # heads 0-3: frontend evacs were plain copies, so apply the b2
        # scale (incl. the denominator ones column) per node tile here
        hbs = []
        for jt in range(NT):
            blk = hb_all[jt][:, h * HB:h * HB + F_OUT + 1]
            if h < 4:
                hb = hbp.tile([P, F_OUT + 1], BF16, name="hb", tag="hb")
                nc.vector.tensor_scalar(hb[:], blk, db[jt][:, 8 + h:9 + h],
                                        None, op0=ALU.mult)
                hbs.append(hb[:])
            else:
                hbs.append(blk)
        us = []
        for jt in range(NT):
            # u = max(exp(0.8 f1_i) * exp(0.8 f2_j), 1)  (one DVE TS, 4x)
            u = tmp_p.tile([P, N], BF16, name="u", tag="u")
            nc.vector.tensor_scalar(u[:], cb[:], db[jt][:, h:h + 1], 1.0,
                                    op0=ALU.mult, op1=ALU.max)
            us.append(u)
        return us, hbs

    def head_attention(h):
        us, hbs = head_prep(h)
        atts = []
        for jt in range(NT):
            att = att_p.tile([P, N], BF16, name="att", tag="att")
            if jt >= NT - att_pool:
                nc.gpsimd.tensor_mul(att[:], us[jt][:], adjTv[:, jt])
            else:
                nc.vector.tensor_mul(att[:], us[jt][:], adjTv[:, jt])
            atts.append(att)
        return atts, hbs

    pair_state = {}

    def epilogue(h, it, atts, hbs):
        op = ps_o.tile([P, F_OUT + 1], F32, name="opsum", tag="opsum")
        for jt in range(NT):
            nc.tensor.matmul(op[:], atts[jt][:, it * P:(it + 1) * P],
                             hbs[jt],
                             start=(jt == 0), stop=(jt == NT - 1))
        rec = ep.tile([P, 1], F32, name="rec", tag="rec")
        nc.vector.reciprocal(rec[:], op[:, F_OUT:F_OUT + 1])
        if it % 2 == 0:
            pair_state["zcp"] = ep.tile([P, 2 * F_OUT], BF16, name="zcp", tag="zcp")
            pair_state["rtp"] = ep.tile([P, 2 * F_OUT], BF16, name="rtp", tag="rtp")
        half = slice((it % 2) * F_OUT, (it % 2 + 1) * F_OUT)
        zt = ep.tile([P, F_OUT], BF16, name="zt", tag="zt")
        nc.scalar.activation(zt[:], op[:, 0:F_OUT], AF.Exp, scale=rec[:, 0:1])
        rtp = pair_state["rtp"]
        if h >= H - rt_pool:
            nc.gpsimd.tensor_scalar(rtp[:, half], op[:, 0:F_OUT], 0.0,
                                    rec[:, 0:1], op0=ALU.max, op1=ALU.mult)
        elif h >= H - rt_dve:
            # tail: ACT is the bottleneck at the end; do relu*rec on DVE
            nc.vector.tensor_scalar(rtp[:, half], op[:, 0:F_OUT], 0.0,
                                    rec[:, 0:1], op0=ALU.max, op1=ALU.mult)
        else:
            nc.scalar.activation(rtp[:, half], op[:, 0:F_OUT], AF.Relu,
                                 scale=rec[:, 0:1])
        nc.gpsimd.tensor_scalar(pair_state["zcp"][:, half], zt[:], 1.0, None,
                                op0=ALU.min)
        if it % 2 == 1:
            # one accumulation chain per PSUM bank: full-width matmuls
            bank = s_ps[it // 2]
            nc.tensor.matmul(bank[:], ident_bf[:], pair_state["zcp"][:],
                             start=(h == 0), stop=False)
            nc.tensor.matmul(bank[:], ident_bf[:], pair_state["rtp"][:],
                             start=False, stop=(h == H - 1))
        if h == H - 1 and it % 2 == 1:
            for i0 in (it - 1, it):
                zz = dp.tile([P, F_OUT], BF16, name="zz", tag="zz")
                nc.scalar.activation(zz[:], s_view(i0), AF.Exp,
                                     accum_out=ds_view(i0))
            if it in (3, 5, NT - 1):
                # stream the log-softmax finale: its 0-3 at it=3 (one DMA),
                # 4-5 at it=5, 6-7 at the end (per-pair DMAs shrink the tail)
                lo, hi = {3: (0, 3), 5: (4, 5), NT - 1: (6, 7)}[it]
                nc.scalar.activation(lnd_st[it][:], ds_st[it][:], AF.Ln)
                for i2 in range(lo, hi + 1):
                    nc.vector.tensor_scalar(
                        ot_all[:, (i2 % 4) * F_OUT:(i2 % 4 + 1) * F_OUT],
                        s_view(i2), lnd_view(i2), None,
                        op0=ALU.subtract)
                odv = out_d.rearrange("(q p) f -> p q f", p=P)
                otv = ot_all[:].rearrange("p (q f) -> p q f", q=4)
                if it == 3:
                    nc.scalar.dma_start(odv[:, 0:4], otv)
                else:
                    q0 = (lo % 4)
                    nc.scalar.dma_start(odv[:, lo:hi + 1], otv[:, q0:q0 + 2])

    def head_block(h, groups=()):
        atts, hbs = head_attention(h)
        # pre-broadcast the next head's exp(0.8 f1) row while this head runs
        if h + 1 < H and h + 1 not in cb_pre:
            emit_cb(h + 1)
        gi = 0
        for it in range(NT):
            epilogue(h, it, atts, hbs)
            # spread stage-B groups (passes 2-3, heads 4-7) thinly across
            # the early heads' epilogue slots: fills PE slack without
            # head-of-line blocking the opsum chains
            if gi < len(groups) and it % 2 == 1:
                hp, nt = groups[gi]
                stage_b_group(hp, nt, pool=ps_h2, fold=True,
                              fold_pool=bool(cfg.get('foldpool', 0)))
                gi += 1
        for hp, nt in groups[gi:]:
            stage_b_group(hp, nt, pool=ps_h2, fold=True,
                          fold_pool=bool(cfg.get('foldpool', 0)))

    def head_block0(groups=()):
        """Head 0 with column-paced attention, interleaved with the adjT
        transposes: the mask-mul for column block `it` only needs adj
        row-tile `it` transposed, so each epilogue(0, it) starts as soon
        as psT(it) lands instead of waiting for the full adjT."""
        adjt_pass([0, 1])
        us, hbs = head_prep(0)
        if 1 not in cb_pre:
            emit_cb(1)
        atts = [att_p.tile([P, N], BF16, name="att", tag="att")
                for _ in range(NT)]
        gi = 0
        for it in range(NT):
            if it + 2 < NT:
                adjt_pass([it + 2])
            cs = slice(it * P, (it + 1) * P)
            for jt in range(NT):
                nc.vector.tensor_mul(atts[jt][:, cs], us[jt][:, cs],
                                     adjTv[:, jt, cs])
            epilogue(0, it, atts, hbs)
            if gi < len(groups) and it % 2 == 1:
                hp, nt = groups[gi]
                stage_b_group(hp, nt, pool=ps_h2, fold=True,
                              fold_pool=bool(cfg.get('foldpool', 0)))
                gi += 1
        for hp, nt in groups[gi:]:
            stage_b_group(hp, nt, pool=ps_h2, fold=True,
                          fold_pool=bool(cfg.get('foldpool', 0)))

    s_ps = [ps_s.tile([P, 2 * F_OUT], F32, name=f"sps{q}", tag=f"sps{q}")
            for q in range(NT // 2)]

    def s_view(it):
        return s_ps[it // 2][:, (it % 2) * F_OUT:(it % 2 + 1) * F_OUT]

    # separate per-finale-stage tiles: avoids false WAR/RAW serialization
    # between the three ln/subtract stages at the tail
    ds_st = {3: dp.tile([P, 4], F32, name="ds_a", tag="ds_a"),
             5: dp.tile([P, 2], F32, name="ds_b", tag="ds_b"),
             7: dp.tile([P, 2], F32, name="ds_c", tag="ds_c")}
    lnd_st = {3: dp.tile([P, 4], F32, name="lnd_a", tag="lnd_a"),
              5: dp.tile([P, 2], F32, name="lnd_b", tag="lnd_b"),
              7: dp.tile([P, 2], F32, name="lnd_c", tag="lnd_c")}

    def ds_view(i0):
        st = 3 if i0 < 4 else (5 if i0 < 6 else 7)
        lo = 0 if i0 < 4 else (4 if i0 < 6 else 6)
        return ds_st[st][:, i0 - lo:i0 - lo + 1]

    def lnd_view(i0):
        st = 3 if i0 < 4 else (5 if i0 < 6 else 7)
        lo = 0 if i0 < 4 else (4 if i0 < 6 else 6)
        return lnd_st[st][:, i0 - lo:i0 - lo + 1]
    op_out = ctx.enter_context(tc.tile_pool(name="outp", bufs=1))
    ot_all = op_out.tile([P, 4 * F_OUT], F32, name="outall", tag="outall")

    g_all = [(2 + g // 8, g % 8) for g in range(16)]
    g_sched = [g_all[0:3], g_all[3:6], g_all[6:9], g_all[9:12], g_all[12:16],
               [], [], []]
    head_block0(groups=g_sched[0])
    for h in range(1, H):
        head_block(h, groups=g_sched[h])



_PROGRAM_CACHE = {}


def build_gat_program(repeats=1, variant=()):
    key = ("nc", repeats, tuple(sorted(variant)))
    if key in _PROGRAM_CACHE:
        return _PROGRAM_CACHE[key]
    if "nopin" not in variant:
        _pin_activation_tables()
    nc = bacc.Bacc("TRN2", debug=False)
    x_d = nc.dram_tensor("x", (N, F_IN), F32, kind="ExternalInput").ap()
    adj_d = nc.dram_tensor("adj", (N, N), I32, kind="ExternalInput").ap()
    W_d = nc.dram_tensor("W", (H, F_OUT, F_IN), F32, kind="ExternalInput").ap()
    a1_d = nc.dram_tensor("a1", (H, F_OUT), F32, kind="ExternalInput").ap()
    a2_d = nc.dram_tensor("a2", (H, F_OUT), F32, kind="ExternalInput").ap()
    out_d = nc.dram_tensor("out", (N, F_OUT), F32, kind="ExternalOutput").ap()
    with tile.TileContext(nc) as tc:
        for _ in range(repeats):
            gat_kernel(tc, out_d, x_d, adj_d, W_d, a1_d, a2_d, variant=variant)
    nc.compile()
    _PROGRAM_CACHE[key] = nc
    return nc


def kernel(x, adj, W, a1, a2, _trace=False, _variant=()):
    from concourse.bass_utils import run_bass_kernel_spmd

    x = np.ascontiguousarray(np.asarray(x, dtype=np.float32))
    adj = np.ascontiguousarray(np.asarray(adj, dtype=np.int32))
    W = np.ascontiguousarray(np.asarray(W, dtype=np.float32))
    a1 = np.ascontiguousarray(np.asarray(a1, dtype=np.float32))
    a2 = np.ascontiguousarray(np.asarray(a2, dtype=np.float32))

    nc = build_gat_program(variant=_variant)
    in_maps = [{"x": x[b], "adj": adj[b], "W": W, "a1": a1, "a2": a2}
               for b in range(B)]
    res = run_bass_kernel_spmd(nc, in_maps, core_ids=list(range(B)),
                               trace=_trace)
    out = np.stack([res.results[b]["out"] for b in range(B)])
    if _trace:
        kernel.last_result = res
    return out


# revision 77
# speedup vs baseline: 2.2512x; 2.2512x over previous
"""GAT (graph attention) Trainium2 kernel.

Full-input contract: kernel(**inputs) takes the unsharded tensors
  x   (8, 1024, 512) f32
  adj (8, 1024, 1024) i32
  W   (8, 256, 512) f32
  a1  (8, 256) f32
  a2  (8, 256) f32
and returns out (8, 1024, 256) f32.

Sharding: data-parallel over batch B=8 across the 8 NeuronCores; each core
computes all heads for one batch element. No collectives needed.

Attention restructuring (v2, exact math): with v = f1_i + f2_j,
  exp(lrelu(v)) = max(e^v, e^{0.2v}) = e^{0.2 f1_i} * [ e^{0.2 f2_j} *
                  max(e^{0.8 (f1_i + f2_j)}, 1) ]
and the e^{0.2 f1_i} row factor cancels in the softmax normalization, so
the unnormalized attention only needs
  t_ij = adj_ij * b2_j * max(c_i * d_j, 1)
with c = exp(0.8 f1) (bf16 row, partition-broadcast), d = exp(0.8 f2) and
b2 = exp(0.2 f2) per-partition scalars. Per (head, node-tile) this is ONE
DVE tensor_scalar (4x mode: u = max(cb*d, 1)) plus ONE adjacency-mask
multiply; b2 is folded into the stage-B PSUM evacuation (tensor_scalar
mult instead of plain copy) and into the denominator ones-column.

Structure (pipelined): DMA order a12 -> x -> W -> adj. W is processed in
head pairs as it arrives (w12, WT transposes, stage-B passes 0-1 with
plain-copy evacs — no f12 dependency); passes 2-3 (heads 4-7, with the b2
fold in the evac) are spread one group per epilogue slot across the early
heads. Head 0's attention is column-paced: its mask-muls and epilogues
start per 128-column block as each adj row-tile is cast + transposed, so
the stream begins during the adj DMA tail. Heads 0-3 apply b2 via
per-head stream tensor_scalars instead of the evac fold. All activations
are pinned to the natural_log_exp_and_others hardware table set (one
LoadActFuncSet total, no exp<->ln thrash at the log-softmax finale).

  h_h   = x @ W_h^T                  (bf16 PE matmul, fp32 accum)
  f1/f2 = x @ (W_h^T a)              (bf16 inputs, fp32 accum)
  o = att @ [b2*h | b2]              (PE; b2-scaled ones col = denom)
  elu(o/den) + 1 = min(exp(o/den),1) + max(o/den,0): exp on ACT, relu on
      ACT/DVE, min-clamp on Pool; the head-sum accumulates on PE via
      identity matmuls into PSUM (one accumulation chain per bank, so
      epilogue tiles are paired to full bank width)
  out = log_softmax(sum_h ...)       finale in three stages (its 0-3 at
      it=3, 4-5 at it=5, 6-7 at the end) with separate ds/lnd tiles per
      stage and per-stage output DMAs on the ACT-issued HWDGE queue
"""
import sys

sys.path.insert(0, "/opt/trn_rl_repo")

from contextlib import ExitStack

import numpy as np

import concourse.bacc as bacc
import concourse.bass as bass
import concourse.mybir as mybir
import concourse.tile as tile
from concourse import masks
from concourse._compat import with_exitstack

# Pin every activation to the one hardware table set that holds all four
# functions we use (exp, relu, copy, ln): empty the other sets so the
# table-load pass must choose it, giving ONE LoadActFuncSet for the whole
# program instead of thrashing between the exp and ln sets around the
# log-softmax finale. Set indices are preserved, so the emitted
# act_func_set_id still matches act_info.json.
_PINNED_ACT_SET = "natural_log_exp_and_others"


def _pin_activation_tables():
    import concourse.hw_specs as hw_specs
    if getattr(bacc, "_gat_act_pin", None):
        return
    orig = bacc.get_activation_tables

    def pinned(arch):
        tabs = orig(arch)
        if _PINNED_ACT_SET not in tabs:
            return tabs
        return {k: (v if k == _PINNED_ACT_SET else set()) for k, v in tabs.items()}

    bacc.get_activation_tables = pinned
    hw_specs.get_activation_tables = pinned
    bacc._gat_act_pin = True

F32 = mybir.dt.float32
BF16 = mybir.dt.bfloat16
I32 = mybir.dt.int32
AF = mybir.ActivationFunctionType
ALU = mybir.AluOpType

N, F_IN, F_OUT, H, B = 1024, 512, 256, 8, 8
P = 128
NT = N // P        # 8 node tiles
FT = F_IN // P     # 4 f_in tiles
OT = F_OUT // P    # 2 f_out tiles
HB = F_OUT + 2     # per-head block in hb_all: 256 values + denom col + pad


@with_exitstack
def gat_kernel(ctx: ExitStack, tc, out_d, x_d, adj_d, W_d, a1_d, a2_d,
               variant=()):
    nc = tc.nc
    cfg = {k.split("=")[0]: int(k.split("=")[1]) for k in variant if "=" in k}
    att_pool = cfg.get("attpool", 0)   # per head: trailing att muls on Pool
    rt_dve = cfg.get("rtdve", 1)       # trailing heads whose rt runs on DVE
    rt_pool = cfg.get("rtpool", 0)     # heads >= H - rtpool: rt on Pool
    adj_act = cfg.get("adjact", 1)     # half the adjT evacs on ACT

    const = ctx.enter_context(tc.tile_pool(name="const", bufs=1))
    ident = const.tile([P, P], F32, name="ident", tag="ident")
    masks.make_identity(nc, ident[:])
    ident_bf = const.tile([P, P], BF16, name="ident_bf", tag="ident_bf")
    masks.make_identity(nc, ident_bf[:])

    persist = ctx.enter_context(tc.tile_pool(name="persist", bufs=1))
    xT_bf = [persist.tile([P, N], BF16, name=f"xTbf{fc}", tag=f"xTbf{fc}") for fc in range(FT)]
    WT_bf = [persist.tile([P, H * F_OUT], BF16, name=f"WTbf{fc}", tag=f"WTbf{fc}") for fc in range(FT)]
    hb_all = [persist.tile([P, H * HB], BF16, name=f"hball{nt}", tag=f"hball{nt}") for nt in range(NT)]
    adjT = persist.tile([P, NT * N], BF16, name="adjT", tag="adjT")
    adjTv = adjT[:].rearrange("p (jt i) -> p jt i", jt=NT)
    f12 = [persist.tile([P, 16], F32, name=f"f12_{nt}", tag=f"f12_{nt}") for nt in range(NT)]
    # db[nt]: cols 0:8 = d = exp(0.8 f2) per head, cols 8:16 = b2 = exp(0.2 f2)
    db = [persist.tile([P, 16], F32, name=f"db_{nt}", tag=f"db_{nt}") for nt in range(NT)]
    e1flat = persist.tile([1, H * N], BF16, name="e1flat", tag="e1flat")

    # pools that outlive stage A must be created before it (LIFO closing)
    pj = ctx.enter_context(tc.tile_pool(name="adjload", bufs=cfg.get("pjb", 5)))
    pjb = ctx.enter_context(tc.tile_pool(name="adjcast", bufs=8))

    # ps_h (frontend stage-B PSUM) lives in its own stack closed right after
    # stage A so the stream region gets its banks back
    psh_stack = ctx.enter_context(ExitStack())
    ps_h = psh_stack.enter_context(
        tc.tile_pool(name="psH", bufs=cfg.get("psh", 2), space="PSUM"))

    sa = ctx.enter_context(ExitStack())  # stage-A transients, closed manually
    pa = sa.enter_context(tc.tile_pool(name="stageA", bufs=8))
    pa2 = sa.enter_context(tc.tile_pool(name="stageA2", bufs=16))
    xtf_pool = sa.enter_context(tc.tile_pool(name="xtf", bufs=1))
    ps_a = sa.enter_context(tc.tile_pool(name="psA", bufs=2, space="PSUM"))
    ps_ft = sa.enter_context(tc.tile_pool(name="psFt", bufs=1, space="PSUM"))
    ps_aw = sa.enter_context(tc.tile_pool(name="psAw", bufs=1, space="PSUM"))
    ps_af = sa.enter_context(tc.tile_pool(name="psAf", bufs=2, space="PSUM"))

    w12_sb = xtf_pool.tile([P, 64], F32, name="w12", tag="w12")
    a12_sb = xtf_pool.tile([16, F_OUT], F32, name="a12", tag="a12")

    # ---- DMA queue order: a1/a2, x, W, adj ----
    nc.sync.dma_start(a12_sb[0:8, :], a1_d[:, :])
    nc.sync.dma_start(a12_sb[8:16, :], a2_d[:, :])

    xnats = []
    for nt in range(NT):
        xnat = pa.tile([P, F_IN], F32, name="xnat", tag="xnat")
        nc.sync.dma_start(xnat[:], x_d[nt * P:(nt + 1) * P, :])
        xnats.append(xnat)

    a12T = xtf_pool.tile([P, 32], F32, name="a12T", tag="a12T")
    for ot in range(OT):
        pt = ps_af.tile([P, 16], F32, name="psA_f", tag="psA_f")
        nc.tensor.matmul(pt[:], a12_sb[:, ot * P:(ot + 1) * P],
                         ident[0:16, 0:16], is_transpose=True)
        nc.vector.tensor_copy(a12T[:, ot * 16:(ot + 1) * 16], pt[:])
    a12Tv = a12T[:].rearrange("p (t c h) -> p t c h", t=2, c=2)

    # x transposes (PE, f32) with a single DVE cast-evac per group
    for ntq in range(0, NT, 4):
        for fc in range(FT):
            pt = ps_a.tile([P, 4 * P], F32, name="psA", tag="psA")
            for d in range(4):
                nc.tensor.matmul(pt[:, d * P:(d + 1) * P],
                                 xnats[ntq + d][:, fc * P:(fc + 1) * P],
                                 ident[:], is_transpose=True)
            nc.vector.tensor_copy(xT_bf[fc][:, ntq * P:(ntq + 4) * P], pt[:])

    def stage_b_group(hp, nt, pool=None, fold=False, fold_pool=False):
        """x @ W for head pair hp, node tile nt.

        fold=True (stream passes, heads 4-7): evac applies the b2 scale per
        head (two tensor_scalars split across DVE/ACT). fold=False (frontend
        passes, heads 0-3): plain pair-wide copy — b2 is applied later by
        per-head stream ops, so these evacs have no f12 dependency and can
        run as soon as W arrives.
        """
        hps = (pool or ps_h).tile([P, 2 * F_OUT], F32, name="hpsum", tag="hpsum")
        for fc in range(FT):
            nc.tensor.matmul(hps[:], xT_bf[fc][:, nt * P:(nt + 1) * P],
                             WT_bf[fc][:, hp * 2 * F_OUT:(hp + 1) * 2 * F_OUT],
                             start=(fc == 0), stop=(fc == FT - 1))
        hv = hb_all[nt][:].rearrange("p (h c) -> p h c", h=H)
        src = hps[:].rearrange("p (d c) -> p d c", d=2)
        if not fold:
            # frontend evacs all on DVE: keeps ACT free for the WT evacs
            # and the e1r chain that gates cb0 / head 0
            nc.vector.tensor_copy(hv[:, 2 * hp:2 * hp + 2, 0:F_OUT], src)
            return
        for dh in range(2):
            h = 2 * hp + dh
            if fold_pool:
                nc.gpsimd.tensor_scalar(hv[:, h, 0:F_OUT], src[:, dh],
                                        db[nt][:, 8 + h:9 + h], None,
                                        op0=ALU.mult)
            elif (dh + nt) % 2 == 0:
                nc.vector.tensor_scalar(hv[:, h, 0:F_OUT], src[:, dh],
                                        db[nt][:, 8 + h:9 + h], None,
                                        op0=ALU.mult)
            else:
                nc.scalar.activation(hv[:, h, 0:F_OUT], src[:, dh],
                                     AF.Copy, scale=db[nt][:, 8 + h:9 + h])

    # softmax denominator columns for heads 0-3: plain ones (the per-head
    # stream tensor_scalar turns them into b2); no dependencies, emit first
    for nt in range(NT):
        hv = hb_all[nt][:].rearrange("p (h c) -> p h c", h=H)
        nc.vector.memset(hv[:, 0:4, F_OUT:F_OUT + 1], 1.0)

    # W pair blocks: loads, w12 = W^T @ [a1 a2], WT transposes, and the
    # frontend stage-B passes (pairs 0-1), all paced by W arrival
    w12v = w12_sb[:].rearrange("p (fc c h) -> p fc c h", fc=FT, c=2)
    wnat_all = {}
    for hp in range(4):
        for dh in range(2):
            h = 2 * hp + dh
            wp = ps_aw.tile([P, 8], F32, name="psA_w", tag="psA_w")
            wnats = []
            for ot in range(OT):
                wnat = pa2.tile([P, F_IN], F32, name="wnat", tag="wnat")
                wnats.append(wnat)
                nc.sync.dma_start(wnat[:], W_d[h, ot * P:(ot + 1) * P, :])
            wnat_all[h] = wnats
            for fc in range(FT):
                for ot in range(OT):
                    nc.tensor.matmul(
                        wp[:, fc * 2:(fc + 1) * 2],
                        wnats[ot][:, fc * P:(fc + 1) * P],
                        a12Tv[:, ot, :, h],
                        start=(ot == 0), stop=(ot == OT - 1))
            nc.vector.tensor_copy(w12v[:, :, :, h], wp[:].rearrange("p (fc c) -> p fc c", fc=FT))
        # WT transposes for this pair (PE), ACT evac
        for fc in range(FT):
            pt = ps_a.tile([P, 4 * P], F32, name="psA", tag="psA")
            for dh in range(2):
                for ot in range(OT):
                    nc.tensor.matmul(
                        pt[:, (dh * 2 + ot) * P:(dh * 2 + ot + 1) * P],
                        wnat_all[2 * hp + dh][ot][:, fc * P:(fc + 1) * P],
                        ident[:], is_transpose=True)
            nc.scalar.copy(
                WT_bf[fc][:, 2 * hp * F_OUT:(2 * hp + 2) * F_OUT], pt[:])
        if hp == 0:
            for nt in range(NT):
                stage_b_group(hp, nt)

    w12bf = xtf_pool.tile([P, 64], BF16, name="w12bf", tag="w12bf")
    nc.vector.tensor_copy(w12bf[:], w12_sb[:])
    w12bv = w12bf[:].rearrange("p (fc c h) -> p fc c h", fc=FT, c=2)

    # adj loads (DMA queue ordered after x and W); casts emitted on Pool
    adj_i32 = []
    for it in range(NT):
        ai = pj.tile([P, N], I32, name="adji", tag="adji")
        nc.sync.dma_start(ai[:], adj_d[it * P:(it + 1) * P, :])
        adj_i32.append(ai)

    # f1/f2 = x @ w12 (bf16 inputs, f32 accum); exp(0.8 f1) rows -> e1flat.
    # Two passes so the ft transposes don't head-of-line block PE on each
    # nt's DVE/ACT chain.
    for nt in range(NT):
        fp = ps_af.tile([P, 16], F32, name="psA_f", tag="psA_f")
        for fc in range(FT):
            nc.tensor.matmul(fp[:], xT_bf[fc][:, nt * P:(nt + 1) * P],
                             w12bv[:, fc], start=(fc == 0), stop=(fc == FT - 1))
        nc.vector.tensor_copy(f12[nt][:], fp[:])
        # d = exp(0.8 f2), b2 = exp(0.2 f2) per-partition scalars (ACT, tiny)
        nc.scalar.activation(db[nt][:, 0:8], f12[nt][:, 8:16], AF.Exp, scale=0.8)
        nc.scalar.activation(db[nt][:, 8:16], f12[nt][:, 8:16], AF.Exp, scale=0.2)
        # denominator columns of heads 4-7 = b2 (their evacs fold b2)
        hv = hb_all[nt][:].rearrange("p (h c) -> p h c", h=H)
        nc.vector.tensor_copy(
            hv[:, 4:8, F_OUT:F_OUT + 1],
            db[nt][:, 12:16].rearrange("p (h c) -> p h c", c=1))
    # f1 transposes + exp, batched 4 node tiles per PSUM bank so the
    # latency chain to e1flat is two rounds, not eight; stage-B pass 1
    # groups fill the PE gaps between rounds
    for q in range(2):
        ft = ps_ft.tile([16, 4 * P], F32, name="psA_ft", tag="psA_ft")
        for d in range(4):
            nc.tensor.matmul(ft[:, d * P:(d + 1) * P], f12[4 * q + d][:],
                             ident[:], is_transpose=True)
        e1r = pa.tile([8, 4 * P], BF16, name="e1r", tag="e1r")
        nc.scalar.activation(e1r[:], ft[0:8, :], AF.Exp, scale=0.8)
        # gather via the ACT-issued HWDGE queue: keeps the SP queue free
        # for the bulk input stream
        nc.scalar.dma_start(
            e1flat[0:1, :].rearrange("a (h n) -> a h n", h=H)
            [:, :, 4 * q * P:4 * (q + 1) * P],
            e1r[:])
        for nt in range(4 * q, 4 * q + 4):
            stage_b_group(1, nt)

    # close stage-A transients, then the frontend stage-B PSUM
    sa.close()
    psh_stack.close()

    # Pool queue: first two adj casts (unblocks the first adjT transposes),
    # then the first head broadcast, then the rest interleaved.
    cbp = ctx.enter_context(tc.tile_pool(name="cbp", bufs=2))
    cb_pre = {}
    adj_bf = []

    def emit_adjb(it):
        ab = pjb.tile([P, N], BF16, name="adjb", tag="adjb")
        nc.gpsimd.tensor_scalar(ab[:], adj_i32[it][:], 0, None, op0=ALU.add)
        adj_bf.append(ab)

    def emit_cb(h):
        cb = cbp.tile([P, N], BF16, name="cb", tag="cb")
        nc.gpsimd.partition_broadcast(cb[:], e1flat[0:1, h * N:(h + 1) * N])
        cb_pre[h] = cb

    # Pool order: four casts (DMA-paced; they gate the adjT transposes),
    # cb0 (gates head 0's u ops; e1flat lands ~when cast 3 finishes),
    # remaining casts, cb1
    for it in range(0, 4):
        emit_adjb(it)
    emit_cb(0)
    for it in range(4, NT):
        emit_adjb(it)
    emit_cb(1)

    # ---------------- stage-C pools ----------------
    # bank budget: ps_t 1 + ps_h2 1 + ps_o 2 + ps_s 4 = 8
    ps_t = ctx.enter_context(tc.tile_pool(name="psT", bufs=1, space="PSUM"))
    ps_h2 = ctx.enter_context(tc.tile_pool(name="psH2", bufs=1, space="PSUM"))
    ps_o = ctx.enter_context(tc.tile_pool(name="psO", bufs=cfg.get("pso", 2), space="PSUM"))
    ps_s = ctx.enter_context(tc.tile_pool(name="psS", bufs=1, space="PSUM"))
    tmp_p = ctx.enter_context(tc.tile_pool(name="tmp", bufs=cfg.get("tmpb", 11)))
    att_p = ctx.enter_context(tc.tile_pool(name="attp", bufs=cfg.get("attb", 13)))
    hbp = ctx.enter_context(tc.tile_pool(name="hbp", bufs=cfg.get("hbb", 16)))
    ep = ctx.enter_context(tc.tile_pool(name="epilogue", bufs=cfg.get("epb", 6)))
    dp = ctx.enter_context(tc.tile_pool(name="lsm", bufs=2))

    def adjt_pass(its):
        for it in its:
            for jq in range(0, NT, 4):
                pt = ps_t.tile([P, 4 * P], BF16, name="psT", tag="psT")
                for d in range(4):
                    nc.tensor.matmul(pt[:, d * P:(d + 1) * P],
                                     adj_bf[it][:, (jq + d) * P:(jq + d + 1) * P],
                                     ident_bf[:], is_transpose=True)
                dst = adjTv[:, jq:jq + 4, it * P:(it + 1) * P]
                src = pt[:].rearrange("p (d c) -> p d c", d=4)
                if (it + jq) % 2 == 0 and adj_act:
                    nc.scalar.copy(dst, src)
                else:
                    nc.vector.tensor_copy(dst, src)

    def head_prep(h):
        """cb broadcast, per-head b2 application (heads 0-3), u rows."""
        if h in cb_pre:
            cb = cb_pre.pop(h)
        else:
            cb = cbp.tile([P, N], BF16, name="cb", tag="cb")
            nc.gpsimd.partition_broadcast(cb[:], e1flat[0:1, h * N:(h + 1) * N])
        # heads 0-3: frontend evacs were plain copies, so apply the b2
        # scale (incl. the denominator ones column) per node tile here
        hbs = []
        for jt in range(NT):
            blk = hb_all[jt][:, h * HB:h * HB + F_OUT + 1]
            if h < 4:
                hb = hbp.tile([P, F_OUT + 1], BF16, name="hb", tag="hb")
                nc.vector.tensor_scalar(hb[:], blk, db[jt][:, 8 + h:9 + h],
                                        None, op0=ALU.mult)
                hbs.append(hb[:])
            else:
                hbs.append(blk)
        us = []
        for jt in range(NT):
            # u = max(exp(0.8 f1_i) * exp(0.8 f2_j), 1)  (one DVE TS, 4x)
            u = tmp_p.tile([P, N], BF16, name="u", tag="u")
            nc.vector.tensor_scalar(u[:], cb[:], db[jt][:, h:h + 1], 1.0,
                                    op0=ALU.mult, op1=ALU.max)
            us.append(u)
        return us, hbs

    def head_attention(h):
        us, hbs = head_prep(h)
        atts = []
        for jt in range(NT):
            att = att_p.tile([P, N], BF16, name="att", tag="att")
            if jt >= NT - att_pool:
                nc.gpsimd.tensor_mul(att[:], us[jt][:], adjTv[:, jt])
            else:
                nc.vector.tensor_mul(att[:], us[jt][:], adjTv[:, jt])
            atts.append(att)
        return atts, hbs

    pair_state = {}

    def epilogue(h, it, atts, hbs):
        op = ps_o.tile([P, F_OUT + 1], F32, name="opsum", tag="opsum")
        for jt in range(NT):
            nc.tensor.matmul(op[:], atts[jt][:, it * P:(it + 1) * P],
                             hbs[jt],
                             start=(jt == 0), stop=(jt == NT - 1))
        rec = ep.tile([P, 1], F32, name="rec", tag="rec")
        nc.vector.reciprocal(rec[:], op[:, F_OUT:F_OUT + 1])
        if it % 2 == 0:
            pair_state["zcp"] = ep.tile([P, 2 * F_OUT], BF16, name="zcp", tag="zcp")
            pair_state["rtp"] = ep.tile([P, 2 * F_OUT], BF16, name="rtp", tag="rtp")
        half = slice((it % 2) * F_OUT, (it % 2 + 1) * F_OUT)
        zt = ep.tile([P, F_OUT], BF16, name="zt", tag="zt")
        nc.scalar.activation(zt[:], op[:, 0:F_OUT], AF.Exp, scale=rec[:, 0:1])
        rtp = pair_state["rtp"]
        if h >= H - rt_pool:
            nc.gpsimd.tensor_scalar(rtp[:, half], op[:, 0:F_OUT], 0.0,
                                    rec[:, 0:1], op0=ALU.max, op1=ALU.mult)
        elif h >= H - rt_dve:
            # tail: ACT is the bottleneck at the end; do relu*rec on DVE
            nc.vector.tensor_scalar(rtp[:, half], op[:, 0:F_OUT], 0.0,
                                    rec[:, 0:1], op0=ALU.max, op1=ALU.mult)
        else:
            nc.scalar.activation(rtp[:, half], op[:, 0:F_OUT], AF.Relu,
                                 scale=rec[:, 0:1])
        nc.gpsimd.tensor_scalar(pair_state["zcp"][:, half], zt[:], 1.0, None,
                                op0=ALU.min)
        if it % 2 == 1:
            # one accumulation chain per PSUM bank: full-width matmuls
            bank = s_ps[it // 2]
            nc.tensor.matmul(bank[:], ident_bf[:], pair_state["zcp"][:],
                             start=(h == 0), stop=False)
            nc.tensor.matmul(bank[:], ident_bf[:], pair_state["rtp"][:],
                             start=False, stop=(h == H - 1))
        if h == H - 1 and it % 2 == 1:
            for i0 in (it - 1, it):
                zz = dp.tile([P, F_OUT], BF16, name="zz", tag="zz")
                nc.scalar.activation(zz[:], s_view(i0), AF.Exp,
                                     accum_out=ds_view(i0))
            if it in (3, 5, NT - 1):
                # stream the log-softmax finale: its 0-3 at it=3 (one DMA),
                # 4-5 at it=5, 6-7 at the end (per-pair DMAs shrink the tail)
                lo, hi = {3: (0, 3), 5: (4, 5), NT - 1: (6, 7)}[it]
                nc.scalar.activation(lnd_st[it][:], ds_st[it][:], AF.Ln)
                for i2 in range(lo, hi + 1):
                    nc.vector.tensor_scalar(
                        ot_all[:, (i2 % 4) * F_OUT:(i2 % 4 + 1) * F_OUT],
                        s_view(i2), lnd_view(i2), None,
                        op0=ALU.subtract)
                odv = out_d.rearrange("(q p) f -> p q f", p=P)
                otv = ot_all[:].rearrange("p (q f) -> p q f", q=4)
                if it == 3:
                    nc.scalar.dma_start(odv[:, 0:4], otv)
                else:
                    q0 = (lo % 4)
                    nc.scalar.dma_start(odv[:, lo:hi + 1], otv[:, q0:q0 + 2])

    def head_block(h, groups=()):
        atts, hbs = head_attention(h)
        # pre-broadcast the next head's exp(0.8 f1) row while this head runs
        if h + 1 < H and h + 1 not in cb_pre:
            emit_cb(h + 1)
        gi = 0
        for it in range(NT):
            epilogue(h, it, atts, hbs)
            # spread stage-B groups (passes 2-3, heads 4-7) thinly across
            # the early heads' epilogue slots: fills PE slack without
            # head-of-line blocking the opsum chains
            if gi < len(groups) and it % 2 == 1:
                hp, nt = groups[gi]
                stage_b_group(hp, nt, pool=ps_h2, fold=True,
                              fold_pool=bool(cfg.get('foldpool', 0)))
                gi += 1
        for hp, nt in groups[gi:]:
            stage_b_group(hp, nt, pool=ps_h2, fold=True,
                          fold_pool=bool(cfg.get('foldpool', 0)))

    def head_block0(groups=()):
        """Head 0 with column-paced attention, interleaved with the adjT
        transposes: the mask-mul for column block `it` only needs adj
        row-tile `it` transposed, so each epilogue(0, it) starts as soon
        as psT(it) lands instead of waiting for the full adjT."""
        adjt_pass([0, 1])
        us, hbs = head_prep(0)
        if 1 not in cb_pre:
            emit_cb(1)
        atts = [att_p.tile([P, N], BF16, name="att", tag="att")
                for _ in range(NT)]
        gi = 0
        for it in range(NT):
            if it + 2 < NT:
                adjt_pass([it + 2])
            cs = slice(it * P, (it + 1) * P)
            for jt in range(NT):
                nc.vector.tensor_mul(atts[jt][:, cs], us[jt][:, cs],
                                     adjTv[:, jt, cs])
            epilogue(0, it, atts, hbs)
            if gi < len(groups) and it % 2 == 1:
                hp, nt = groups[gi]
                stage_b_group(hp, nt, pool=ps_h2, fold=True,
                              fold_pool=bool(cfg.get('foldpool', 0)))
                gi += 1
        for hp, nt in groups[gi:]:
            stage_b_group(hp, nt, pool=ps_h2, fold=True,
                          fold_pool=bool(cfg.get('foldpool', 0)))

    s_ps = [ps_s.tile([P, 2 * F_OUT], F32, name=f"sps{q}", tag=f"sps{q}")
            for q in range(NT // 2)]

    def s_view(it):
        return s_ps[it // 2][:, (it % 2) * F_OUT:(it % 2 + 1) * F_OUT]

    # separate per-finale-stage tiles: avoids false WAR/RAW serialization
    # between the three ln/subtract stages at the tail
    ds_st = {3: dp.tile([P, 4], F32, name="ds_a", tag="ds_a"),
             5: dp.tile([P, 2], F32, name="ds_b", tag="ds_b"),
             7: dp.tile([P, 2], F32, name="ds_c", tag="ds_c")}
    lnd_st = {3: dp.tile([P, 4], F32, name="lnd_a", tag="lnd_a"),
              5: dp.tile([P, 2], F32, name="lnd_b", tag="lnd_b"),
              7: dp.tile([P, 2], F32, name="lnd_c", tag="lnd_c")}

    def ds_view(i0):
        st = 3 if i0 < 4 else (5 if i0 < 6 else 7)
        lo = 0 if i0 < 4 else (4 if i0 < 6 else 6)
        return ds_st[st][:, i0 - lo:i0 - lo + 1]

    def lnd_view(i0):
        st = 3 if i0 < 4 else (5 if i0 < 6 else 7)
        lo = 0 if i0 < 4 else (4 if i0 < 6 else 6)
        return lnd_st[st][:, i0 - lo:i0 - lo + 1]
    op_out = ctx.enter_context(tc.tile_pool(name="outp", bufs=1))
    ot_all = op_out.tile([P, 4 * F_OUT], F32, name="outall", tag="outall")

    g_all = [(2 + g // 8, g % 8) for g in range(16)]
    g_sched = [g_all[0:3], g_all[3:6], g_all[6:9], g_all[9:12], g_all[12:16],
               [], [], []]
    head_block0(groups=g_sched[0])
    for h in range(1, H):
        head_block(h, groups=g_sched[h])



_PROGRAM_CACHE = {}


def build_gat_program(repeats=1, variant=()):
    key = ("nc", repeats, tuple(sorted(variant)))
    if key in _PROGRAM_CACHE:
        return _PROGRAM_CACHE[key]
    if "nopin" not in variant:
        _pin_activation_tables()
    nc = bacc.Bacc("TRN2", debug=False)
    x_d = nc.dram_tensor("x", (N, F_IN), F32, kind="ExternalInput").ap()
    adj_d = nc.dram_tensor("adj", (N, N), I32, kind="ExternalInput").ap()
    W_d = nc.dram_tensor("W", (H, F_OUT, F_IN), F32, kind="ExternalInput").ap()
    a1_d = nc.dram_tensor("a1", (H, F_OUT), F32, kind="ExternalInput").ap()
    a2_d = nc.dram_tensor("a2", (H, F_OUT), F32, kind="ExternalInput").ap()
    out_d = nc.dram_tensor("out", (N, F_OUT), F32, kind="ExternalOutput").ap()
    with tile.TileContext(nc) as tc:
        for _ in range(repeats):
            gat_kernel(tc, out_d, x_d, adj_d, W_d, a1_d, a2_d, variant=variant)
    nc.compile()
    _PROGRAM_CACHE[key] = nc
    return nc


def kernel(x, adj, W, a1, a2, _trace=False, _variant=()):
    from concourse.bass_utils import run_bass_kernel_spmd

    x = np.ascontiguousarray(np.asarray(x, dtype=np.float32))
    adj = np.ascontiguousarray(np.asarray(adj, dtype=np.int32))
    W = np.ascontiguousarray(np.asarray(W, dtype=np.float32))
    a1 = np.ascontiguousarray(np.asarray(a1, dtype=np.float32))
    a2 = np.ascontiguousarray(np.asarray(a2, dtype=np.float32))

    nc = build_gat_program(variant=_variant)
    in_maps = [{"x": x[b], "adj": adj[b], "W": W, "a1": a1, "a2": a2}
               for b in range(B)]
    res = run_bass_kernel_spmd(nc, in_maps, core_ids=list(range(B)),
                               trace=_trace)
    out = np.stack([res.results[b]["out"] for b in range(B)])
    if _trace:
        kernel.last_result = res
    return out
